# revision 71
# baseline (speedup 1.0000x reference)
"""Trainium2 Bass kernel for nn_AttentionLayer (B=32, C=512, HW=1024).

Strategy: data-parallel over batch across 8 NeuronCores (4 samples each).
BatchNorm batch-stats (mean/var over batch+spatial) are computed as
per-core partial sums + a tiny 8-core AllGather + local sum, twice (BN1
on x, BN2 on xr = x + attention).  A dummy warmup collective issued at
kernel start absorbs the cc-stream bootstrap barrier (which otherwise
delays AG1 by ~25us).  Attention matmuls run on TensorE in fp8e4m3 with
DoubleRow K-packing (the systematic fp8 error of Wv is cancelled by a
per-channel bias dWv@mean(h), exploiting sum_q softmax == 1).  Softmax
is over the query axis; Z = col-sums of E are computed with a DoubleRow
fp8 ones-matmul directly over the fp8 E tiles.  xr is kept resident in
SBUF (overwrites x in place; no DRAM spill).  The MLP runs in fp8
DoubleRow: the BN2 scale a2 is folded into W1 on device
(W1' = fp8(W1*a2), b1' = b1 + W1@d2) so the MLP consumes an fp8 copy of
xr cast during the attention pass; the residual path stays fp32.

kernel(**inputs) takes the FULL unsharded inputs and returns the FULL
output; sharding/unsharding happens on the host inside this function.
"""

import numpy as np

B, C, HW = 32, 512, 1024
D = C // 8            # 64
N_CORES = 8
B_LOC = B // N_CORES  # 4
P = 128
CO = C // P           # 4
NTOT = float(B * HW)  # BN normalizer (biased stats over batch+spatial)
EPS = 1e-5

_CACHE = {}


def _build_nc():
    import concourse.bass as bass
    import concourse.mybir as mybir
    import concourse.tile as tile
    from concourse import bacc
    from concourse.bass import ts

    f32 = mybir.dt.float32
    bf16 = mybir.dt.bfloat16
    f8 = mybir.dt.float8e4
    PM = mybir.MatmulPerfMode
    AF = mybir.ActivationFunctionType
    ALU = mybir.AluOpType
    AX = mybir.AxisListType

    nc = bacc.Bacc("TRN2", target_bir_lowering=False, debug=False,
                   num_devices=N_CORES)

    # ---------------- I/O ----------------
    x_d = nc.dram_tensor("x", [B_LOC, C, HW], f32, kind="ExternalInput")
    wq_d = nc.dram_tensor("wq_t", [P, CO, D], f8, kind="ExternalInput")
    wk_d = nc.dram_tensor("wk_t", [P, CO, D], f8, kind="ExternalInput")
    wv_d = nc.dram_tensor("wv_t", [P, CO, C], f8, kind="ExternalInput")
    dwv_d = nc.dram_tensor("dwv_t", [P, CO, C], bf16, kind="ExternalInput")
    w1_d = nc.dram_tensor("w1_t", [P, CO, C], bf16, kind="ExternalInput")
    w2_d = nc.dram_tensor("w2_t", [P, CO, C], f8, kind="ExternalInput")
    bq_d = nc.dram_tensor("bq_t", [P, 1], f32, kind="ExternalInput")
    bk_d = nc.dram_tensor("bk_t", [P, 1], f32, kind="ExternalInput")
    bv_d = nc.dram_tensor("bv_t", [P, CO], f32, kind="ExternalInput")
    b1_d = nc.dram_tensor("b1_t", [P, CO], f32, kind="ExternalInput")
    b2_d = nc.dram_tensor("b2_t", [P, CO], f32, kind="ExternalInput")
    g1_d = nc.dram_tensor("g1_t", [P, CO], f32, kind="ExternalInput")
    be1_d = nc.dram_tensor("be1_t", [P, CO], f32, kind="ExternalInput")
    g2_d = nc.dram_tensor("g2_t", [P, CO], f32, kind="ExternalInput")
    be2_d = nc.dram_tensor("be2_t", [P, CO], f32, kind="ExternalInput")
    ones8_d = nc.dram_tensor("ones8_t", [P, 2, P], f8, kind="ExternalInput")
    out_d = nc.dram_tensor("out", [B_LOC, C, HW], f32, kind="ExternalOutput")

    def chw_view(dram3, s):
        # [C, HW] sample -> [P, CO, HW] partition view (c = co*P + p)
        return dram3[s].rearrange("(co p) hw -> p co hw", p=P)

    with tile.TileContext(nc) as tc:
        with (
            tc.tile_pool(name="const", bufs=1) as cpool,
            tc.tile_pool(name="stats", bufs=1) as spool,
            tc.tile_pool(name="dram", bufs=1, space="DRAM") as dpool,
            tc.tile_pool(name="psum", bufs=1, space="PSUM") as ppool,
        ):
            # DRAM scratch for the collectives
            cc1i_d = dpool.tile([P, 2 * CO], f32)
            cc1o_d = dpool.tile([N_CORES * P, 2 * CO], f32)
            cc2i_d = dpool.tile([P, 2 * CO], f32)
            cc2o_d = dpool.tile([N_CORES * P, 2 * CO], f32)

            # ---------- persistent weights ----------
            wq = cpool.tile([P, CO, D], f8)
            wk = cpool.tile([P, CO, D], f8)
            wv = cpool.tile([P, CO, C], f8)
            dwv = cpool.tile([P, CO, C], bf16)
            w1 = cpool.tile([P, CO, C], bf16)
            w1p = cpool.tile([P, CO, C], f8)
            w2 = cpool.tile([P, CO, C], f8)
            bq = cpool.tile([P, 1], f32)
            bk = cpool.tile([P, 1], f32)
            bv = cpool.tile([P, CO], f32)
            b1 = cpool.tile([P, CO], f32)
            b1p = cpool.tile([P, CO], f32)
            b2 = cpool.tile([P, CO], f32)
            g1 = cpool.tile([P, CO], f32)
            be1 = cpool.tile([P, CO], f32)
            g2 = cpool.tile([P, CO], f32)
            be2 = cpool.tile([P, CO], f32)
            ones8 = cpool.tile([P, 2, P], f8)
            eps_t = cpool.tile([P, 1], f32)
            d2b = cpool.tile([P, CO], bf16)
            nc.gpsimd.memset(eps_t[:], EPS)

            # ---------- stats tiles ----------
            ssum1 = spool.tile([P, CO, B_LOC], f32)
            ssq1 = spool.tile([P, CO, B_LOC], f32)
            ssum2 = spool.tile([P, CO, B_LOC], f32)
            ssq2 = spool.tile([P, CO, B_LOC], f32)
            ccin1 = spool.tile([P, 2 * CO], f32)
            ccin2 = spool.tile([P, 2 * CO], f32)
            a1 = spool.tile([P, CO], f32)
            d1 = spool.tile([P, CO], f32)
            a2 = spool.tile([P, CO], f32)
            d2 = spool.tile([P, CO], f32)
            mtmp = spool.tile([P, CO], f32)
            vtmp = spool.tile([P, CO], f32)
            ttmp = spool.tile([P, CO], f32)
            agt = spool.tile([P, 2 * CO], f32)
            agp = spool.tile([P, 4, 2 * CO], f32)
            ag1 = spool.tile([P, N_CORES, 2 * CO], f32)
            ag2 = spool.tile([P, N_CORES, 2 * CO], f32)

            def pack_stats(ccin_sb, cci_d, ssum, ssq):
                """partial sums -> packed DRAM collective input (issued on
                the Scalar queue so they never sit behind bulk spills)"""
                nc.vector.tensor_reduce(ccin_sb[:, 0:CO, None], ssum[:],
                                        axis=AX.X, op=ALU.add)
                nc.scalar.dma_start(cci_d[:, 0:CO], ccin_sb[:, 0:CO])
                nc.vector.tensor_reduce(ccin_sb[:, CO:2 * CO, None], ssq[:],
                                        axis=AX.X, op=ALU.add)
                nc.scalar.dma_start(cci_d[:, CO:2 * CO],
                                    ccin_sb[:, CO:2 * CO])

            def bn_coeffs(cci_d, cco_d, ag_sb, gg, bb, aa, dd):
                """AllGather -> local tree-sum -> a = g*rsqrt(var+eps),
                d = b - mean*a"""
                nc.gpsimd.collective_compute(
                    "AllGather", ALU.bypass,
                    replica_groups=[list(range(N_CORES))],
                    ins=[cci_d[:].opt()], outs=[cco_d[:].opt()],
                )
                nc.scalar.dma_start(
                    ag_sb[:],
                    cco_d[:].rearrange("(r p) f -> p r f", p=P))
                for rr in range(4):
                    nc.vector.tensor_add(agp[:, rr, :], ag_sb[:, 2 * rr, :],
                                         ag_sb[:, 2 * rr + 1, :])
                nc.vector.tensor_add(agp[:, 0, :], agp[:, 0, :],
                                     agp[:, 1, :])
                nc.vector.tensor_add(agp[:, 2, :], agp[:, 2, :],
                                     agp[:, 3, :])
                nc.vector.tensor_add(agt[:], agp[:, 0, :], agp[:, 2, :])
                nc.vector.tensor_scalar_mul(mtmp[:], agt[:, 0:CO],
                                            1.0 / NTOT)
                nc.vector.tensor_scalar_mul(vtmp[:], agt[:, CO:2 * CO],
                                            1.0 / NTOT)
                nc.vector.tensor_mul(ttmp[:], mtmp[:], mtmp[:])
                nc.vector.tensor_sub(vtmp[:], vtmp[:], ttmp[:])
                nc.scalar.activation(vtmp[:], vtmp[:], AF.Sqrt, bias=eps_t[:])
                nc.vector.reciprocal(ttmp[:], vtmp[:])
                nc.vector.tensor_mul(aa[:], gg[:], ttmp[:])
                nc.vector.tensor_mul(ttmp[:], mtmp[:], aa[:])
                nc.vector.tensor_sub(dd[:], bb[:], ttmp[:])

            with tc.tile_pool(name="xp", bufs=1) as xpool:
                # x_all holds x during pass 1/2 and is overwritten in place
                # by xr = x + att during pass 2 (no DRAM spill).  xq_all is
                # the fp8 copy of xr the MLP GEMMs consume.
                x_all = xpool.tile([P, B_LOC, CO, HW], f32)
                xq_all = xpool.tile([P, B_LOC, CO, HW], f8)

                # ============ pass 1: BN1 stats over x ============
                with tc.tile_pool(name="p1", bufs=2) as w1pool:
                    for s in range(B_LOC):
                        for co in range(CO):
                            nc.sync.dma_start(
                                x_all[:, s, co:co + 1, :],
                                chw_view(x_d, s)[:, co:co + 1, :])
                            sq = w1pool.tile([P, HW], f32, tag="sq1")
                            nc.vector.tensor_reduce(
                                ssum1[:, co, s:s + 1], x_all[:, s, co, :],
                                axis=AX.X, op=ALU.add)
                            nc.scalar.activation(
                                sq[:], x_all[:, s, co, :], AF.Square,
                                accum_out=ssq1[:, co, s:s + 1])

                # weight/bias loads (issued after the x DMAs on purpose)
                for t, d in [(wq, wq_d), (wk, wk_d), (wv, wv_d),
                             (dwv, dwv_d), (w1, w1_d),
                             (w2, w2_d), (bq, bq_d), (bk, bk_d), (bv, bv_d),
                             (b1, b1_d), (b2, b2_d), (g1, g1_d),
                             (be1, be1_d), (g2, g2_d), (be2, be2_d),
                             (ones8, ones8_d)]:
                    nc.sync.dma_start(t[:], d[:])

                pack_stats(ccin1, cc1i_d, ssum1, ssq1)
                bn_coeffs(cc1i_d, cc1o_d, ag1, g1, be1, a1, d1)

                # ======== pass 2: attention, xr = x + att (in SBUF) ======
                with tc.tile_pool(name="p2b", bufs=2) as bpool:
                    # finish_row work of sample s-1 is deferred into sample
                    # s's prologue so the scalar/vector queues don't block
                    # the next sample's h/qk/vt on the epilogue.
                    pending_finish = []

                    def flush_finish():
                        while pending_finish:
                            pending_finish.pop(0)()

                    for s in range(B_LOC):
                        xt = x_all[:, s]
                        qz = bpool.tile([P, HW], bf16, tag="qz")
                        kz = bpool.tile([P, HW], bf16, tag="kz")

                        # h = relu(a1*x + d1); hsum = row sums for the
                        # fp8-Wv DC correction (sum_q E/Z == 1 exactly, so
                        # the fp8 weight-rounding error folds into a
                        # per-channel bias dWv @ mean_q(h))
                        h = bpool.tile([P, CO, HW], f8, tag="h", bufs=3)
                        hsum = bpool.tile([P, CO], f32, tag="hsum")
                        for co in range(CO):
                            nc.scalar.activation(h[:, co, :], xt[:, co, :],
                                                 AF.Relu,
                                                 bias=d1[:, co:co + 1],
                                                 scale=a1[:, co:co + 1],
                                                 accum_out=hsum[:, co:co + 1])
                        # q = Wq @ h + bq, k = Wk @ h + bk ([D, HW], rows
                        # 0..63 of the qz/kz tiles).  Both hw-halves land
                        # in one 2-bank psum tile so a single wide
                        # activation copies each of q/k out.
                        for wgt, bias_t, dst in ((wq, bq, qz), (wk, bk, kz)):
                            qkp = ppool.tile([P, 2, 512], f32, tag="psB",
                                             bufs=2, name="qkp")
                            for n2 in range(2):
                                for c2 in range(2):
                                    nc.tensor.matmul(
                                        qkp[0:D, n2, :],
                                        wgt[:, 2 * c2:2 * c2 + 2, :],
                                        h[:, 2 * c2:2 * c2 + 2,
                                          ts(n2, 512)],
                                        start=(c2 == 0), stop=(c2 == 1),
                                        perf_mode=PM.DoubleRow)
                            nc.scalar.activation(dst[0:D, :],
                                                 qkp[0:D, :, :],
                                                 AF.Identity,
                                                 bias=bias_t[0:D, :])

                        # hm for the DC correction (used after z below)
                        hm = bpool.tile([P, CO], bf16, tag="hm")
                        nc.vector.tensor_scalar_mul(hm[:], hsum[:], 1.0 / HW)

                        # beta+exp interleaved with vT: each beta (be,bo)
                        # j-pair lands in one 2-bank psum tile consumed by a
                        # single paired exp; vT matmul pairs fill the tensor
                        # queue between beta pairs so the exp latency never
                        # blocks it (tagB bufs=2 keeps 2 pairs in flight).
                        vt = bpool.tile([P, 8, C], f8, tag="vt", bufs=3)
                        E = bpool.tile([P, 8, HW], f8, tag="E", bufs=3)
                        lo = slice(0, D)
                        for k8 in range(8):
                            j2, n2 = k8 // 2, k8 % 2
                            je, jo = 2 * j2, 2 * j2 + 1
                            bp = ppool.tile([P, 2, 512], f32, tag="psB",
                                            bufs=2)
                            nc.tensor.matmul(bp[:, 0, :],
                                             qz[lo, ts(je, P)],
                                             kz[lo, ts(n2, 512)],
                                             start=True, stop=True)
                            nc.tensor.matmul(bp[:, 1, :],
                                             qz[lo, ts(jo, P)],
                                             kz[lo, ts(n2, 512)],
                                             start=True, stop=True)
                            nc.scalar.activation(
                                E[:, je:je + 2, ts(n2, 512)], bp[:],
                                AF.Exp, scale=0.125)
                            jw = k8
                            vtps = ppool.tile([P, 512], f32, tag="ps512",
                                              bufs=4)
                            for c2 in range(2):
                                nc.tensor.matmul(
                                    vtps[:],
                                    h[:, 2 * c2:2 * c2 + 2, ts(jw, P)],
                                    wv[:, 2 * c2:2 * c2 + 2, :],
                                    start=(c2 == 0), stop=(c2 == 1),
                                    perf_mode=PM.DoubleRow)
                            nc.vector.tensor_copy(vt[:, jw, :], vtps[:])

                        # epilogue of the previous sample (cast + squares);
                        # the casts queue on Scalar after this sample's exps
                        flush_finish()

                        # Z = col-sums of E via DoubleRow fp8 ones-matmul
                        # (replicated across all 128 partitions), then a
                        # fast reciprocal on DVE.
                        rz = bpool.tile([P, HW], f32, tag="rz")
                        for n2 in range(2):
                            zps = ppool.tile([P, 512], f32,
                                             tag="ps512", bufs=4)
                            for jp in range(4):
                                nc.tensor.matmul(
                                    zps[:],
                                    ones8[:],
                                    E[:, 2 * jp:2 * jp + 2, ts(n2, 512)],
                                    start=(jp == 0), stop=(jp == 3),
                                    perf_mode=PM.DoubleRow)
                            nc.vector.reciprocal_approx_fast(
                                out=rz[:, ts(n2, 512)], in_=zps[:])

                        # fp8-Wv DC correction bias
                        cps = ppool.tile([P, 512], f32, tag="ps512", bufs=4)
                        for mo in range(CO):
                            for ci in range(CO):
                                nc.tensor.matmul(cps[:, mo:mo + 1],
                                                 dwv[:, ci, ts(mo, P)],
                                                 hm[:, ci, None],
                                                 start=(ci == 0),
                                                 stop=(ci == 3))
                        biasn = bpool.tile([P, CO], f32, tag="biasn")
                        nc.vector.tensor_add(biasn[:], cps[:, 0:CO], bv[:])

                        # att = (v @ E) / Z ; xr = x + att + bias (in place)
                        aps_tiles = {}
                        attsum = bpool.tile([P, CO, 2], f32, tag="attsum")

                        def att_group(mo, n2, k=[0]):
                            # alternate the psum ring between ps512 and the
                            # beta tiles (idle during the att stretch) for
                            # an effective 6-deep ring
                            if k[0] % 2 == 0:
                                aps = ppool.tile([P, 512], f32, tag="ps512",
                                                 bufs=4, name="apsA")
                            else:
                                apb = ppool.tile([P, 2, 512], f32,
                                                 tag="psB", bufs=2,
                                                 name="apsB")
                                aps = apb[:, 0, :]
                            k[0] += 1
                            for j4 in range(4):
                                nc.tensor.matmul(
                                    aps[:],
                                    vt[:, 2 * j4:2 * j4 + 2, ts(mo, P)],
                                    E[:, 2 * j4:2 * j4 + 2, ts(n2, 512)],
                                    start=(j4 == 0), stop=(j4 == 3),
                                    perf_mode=PM.DoubleRow)
                            aps_tiles[(mo, n2)] = aps

                        last_s = (s == B_LOC - 1)

                        def consume(mo, n2):
                            aps = aps_tiles.pop((mo, n2))
                            xsl = xt[:, mo, ts(n2, 512)]
                            tmp = bpool.tile([P, 512], f32, tag="tmp",
                                             bufs=6)
                            nc.vector.affine_mul_reduce(
                                out=tmp[:],
                                accum_out=attsum[:, mo, n2:n2 + 1],
                                in0=aps[:], in1=rz[:, ts(n2, 512)],
                                scale=1.0, bias=0.0)
                            nc.vector.scalar_tensor_tensor(
                                out=xsl, in0=tmp[:],
                                scalar=biasn[:, mo:mo + 1], in1=xsl,
                                op0=ALU.add, op1=ALU.add)

                        def cast_row(mo, s=s, xt=xt):
                            # fp8 copy of the xr row for the MLP.  Runs on
                            # Scalar, which idles during the att stretch.
                            # (BN2 ssum comes from the attsum accumulators:
                            # sum(xr) = sum(x) + sum(att) + HW*biasn.)
                            nc.scalar.activation(
                                xq_all[:, s, mo, :], xt[:, mo, :],
                                AF.Identity)

                        def sq_row_scalar(mo, s=s, xt=xt):
                            sq = bpool.tile([P, HW], f32, tag="sq2",
                                            bufs=3)
                            nc.scalar.activation(
                                sq[:], xt[:, mo, :], AF.Square,
                                accum_out=ssq2[:, mo, s:s + 1])

                        def sq_row_dve(mo, s=s, xt=xt):
                            sq = bpool.tile([P, HW], f32, tag="sq2",
                                            bufs=3)
                            nc.vector.affine_mul_reduce(
                                out=sq[:],
                                accum_out=ssq2[:, mo, s:s + 1],
                                in0=xt[:, mo, :], in1=xt[:, mo, :],
                                scale=1.0, bias=0.0)

                        groups = [(mo, n2) for mo in range(CO)
                                  for n2 in range(2)]
                        lag = 2 if last_s else 3
                        for idx, g in enumerate(groups):
                            att_group(*g)
                            if idx >= lag:
                                consume(*groups[idx - lag])
                                done = groups[idx - lag]
                                if done[1] == 1:
                                    # row done: fp8 cast on Scalar (idle
                                    # during the att stretch)
                                    cast_row(done[0])
                                    if last_s:
                                        if done[0] < 2:
                                            sq_row_scalar(done[0])
                                        else:
                                            sq_row_dve(done[0])
                        for g in groups[-lag:]:
                            consume(*g)
                            if g[1] == 1:
                                cast_row(g[0])
                                if last_s:
                                    if g[0] < 2:
                                        sq_row_scalar(g[0])
                                    else:
                                        sq_row_dve(g[0])
                        # ssum2 = sum(x) + sum(att) + HW*biasn  (tiny DVE)
                        atot = bpool.tile([P, CO], f32, tag="atot")
                        nc.vector.tensor_reduce(atot[:, :, None], attsum[:],
                                                axis=AX.X, op=ALU.add)
                        nc.vector.tensor_add(atot[:], atot[:],
                                             ssum1[:, :, s])
                        nc.vector.tensor_scalar(ssum2[:, :, s], biasn[:],
                                                float(HW), None,
                                                ALU.mult, ALU.bypass)
                        nc.vector.tensor_add(ssum2[:, :, s], ssum2[:, :, s],
                                             atot[:])

                        if last_s:
                            pack_stats(ccin2, cc2i_d, ssum2, ssq2)
                        else:
                            # squares go to DVE in the next sample's
                            # prologue (DVE has slack there)
                            for mo in range(CO):
                                pending_finish.append(
                                    lambda mo=mo, fr=sq_row_dve: fr(mo))

                bn_coeffs(cc2i_d, cc2o_d, ag2, g2, be2, a2, d2)

                # === pass 3: MLP, out = xr + W2 relu(W1 bn2(xr) + b1) + b2
                # BN2 folded into the weights: W1' = fp8(W1 * a2) (per input
                # channel), b1' = b1 + W1 @ d2, so the GEMMs read the fp8 xr
                # copy made during pass 2.
                nc.vector.tensor_copy(d2b[:], d2[:])
                for ci in range(CO):
                    nc.scalar.activation(w1p[:, ci, :], w1[:, ci, :],
                                         AF.Identity, scale=a2[:, ci:ci + 1])
                cpsB = ppool.tile([P, 512], f32, tag="ps512", bufs=4)
                for mo in range(CO):
                    for ci in range(CO):
                        nc.tensor.matmul(cpsB[:, mo:mo + 1],
                                         w1[:, ci, ts(mo, P)],
                                         d2b[:, ci, None],
                                         start=(ci == 0), stop=(ci == 3))
                nc.vector.tensor_add(b1p[:], cpsB[:, 0:CO], b1[:])

                with tc.tile_pool(name="p3", bufs=2) as mpool:
                    # software-pipelined: y2 of sample s-1 interleaves with
                    # y1 of sample s so the tensor queue never waits for the
                    # relu (scalar) or residual-add (DVE) consumers.
                    y1_tiles = {}

                    def y1_step(s, mo):
                        xq = xq_all[:, s]
                        if s not in y1_tiles:
                            y1_tiles[s] = mpool.tile([P, CO, HW], f8,
                                                     tag="y1", name="y1t")
                        y1 = y1_tiles[s]
                        yps = ppool.tile([P, 2, 512], f32, tag="psB",
                                         bufs=2, name="y1ps")
                        for n2 in range(2):
                            for cp in range(2):
                                nc.tensor.matmul(
                                    yps[:, n2, :],
                                    w1p[:, 2 * cp:2 * cp + 2, ts(mo, P)],
                                    xq[:, 2 * cp:2 * cp + 2, ts(n2, 512)],
                                    start=(cp == 0), stop=(cp == 1),
                                    perf_mode=PM.DoubleRow)
                        nc.scalar.activation(
                            y1[:, mo, :].rearrange("p (n c) -> p n c",
                                                   n=2),
                            yps[:], AF.Relu, bias=b1p[:, mo:mo + 1])

                    def y2_step(s, mo):
                        y1 = y1_tiles[s]
                        yps = ppool.tile([P, 2, 512], f32, tag="psB",
                                         bufs=2, name="y2ps")
                        for n2 in range(2):
                            for cp in range(2):
                                nc.tensor.matmul(
                                    yps[:, n2, :],
                                    w2[:, 2 * cp:2 * cp + 2, ts(mo, P)],
                                    y1[:, 2 * cp:2 * cp + 2, ts(n2, 512)],
                                    start=(cp == 0), stop=(cp == 1),
                                    perf_mode=PM.DoubleRow)
                        ot = mpool.tile([P, HW], f32, tag="ot", bufs=3)
                        nc.vector.affine_then_add(
                            out=ot[:],
                            in0=yps[:].rearrange("p n c -> p (n c)"),
                            in1=x_all[:, s, mo, :],
                            scale=1.0, bias=b2[:, mo:mo + 1])
                        nc.sync.dma_start(
                            chw_view(out_d, s)[:, mo, :], ot[:])

                    for mo in range(CO):
                        y1_step(0, mo)
                    for s in range(1, B_LOC):
                        for mo in range(CO):
                            y2_step(s - 1, mo)
                            y1_step(s, mo)
                        del y1_tiles[s - 1]
                    for mo in range(CO):
                        y2_step(B_LOC - 1, mo)

    nc.compile()
    return nc


def _prep_in_maps(inputs):
    import ml_dtypes
    bf = ml_dtypes.bfloat16
    f8 = ml_dtypes.float8_e4m3
    x = np.ascontiguousarray(inputs["x"], dtype=np.float32)
    wqkv = np.asarray(inputs["W_qkv"], dtype=np.float32)
    bqkv = np.asarray(inputs["b_qkv"], dtype=np.float32)

    def chan_t(w, dt=bf):  # [O, C] -> [P, CO, O]
        w = np.asarray(w, dtype=np.float32)
        o = w.shape[0]
        return np.ascontiguousarray(
            w.reshape(o, CO, P).transpose(2, 1, 0).astype(dt))

    def vec_t(v):  # [C] -> [P, CO]
        return np.ascontiguousarray(
            np.asarray(v, dtype=np.float32).reshape(CO, P).T)

    shared = {
        "wq_t": chan_t(wqkv[:D], f8),
        "wk_t": chan_t(wqkv[D:2 * D], f8),
        "wv_t": chan_t(wqkv[2 * D:], f8),
        "dwv_t": chan_t(wqkv[2 * D:]
                        - wqkv[2 * D:].astype(f8).astype(np.float32)),
        "w1_t": chan_t(inputs["W1"]),
        "w2_t": chan_t(inputs["W2"], f8),
        "bq_t": np.ascontiguousarray(
            np.concatenate([bqkv[:D], np.zeros(D)])[:, None],
            dtype=np.float32),
        "bk_t": np.ascontiguousarray(
            np.concatenate([bqkv[D:2 * D], np.zeros(D)])[:, None],
            dtype=np.float32),
        "bv_t": vec_t(bqkv[2 * D:]),
        "b1_t": vec_t(inputs["b1"]),
        "b2_t": vec_t(inputs["b2"]),
        "g1_t": vec_t(inputs["bn1_g"]),
        "be1_t": vec_t(inputs["bn1_b"]),
        "g2_t": vec_t(inputs["bn2_g"]),
        "be2_t": vec_t(inputs["bn2_b"]),
        "ones8_t": np.ones((P, 2, P), dtype=f8),
    }
    in_maps = []
    for c in range(N_CORES):
        m = dict(shared)
        m["x"] = np.ascontiguousarray(x[c * B_LOC:(c + 1) * B_LOC])
        in_maps.append(m)
    return in_maps


def kernel_with_results(inputs, trace=False):
    from concourse import bass_utils
    if "nc" not in _CACHE:
        _CACHE["nc"] = _build_nc()
    nc = _CACHE["nc"]
    in_maps = _prep_in_maps(inputs)
    res = bass_utils.run_bass_kernel_spmd(
        nc, in_maps, core_ids=list(range(N_CORES)), trace=trace)
    out = np.concatenate([res.results[c]["out"] for c in range(N_CORES)],
                         axis=0)
    return out, res


def kernel(**inputs):
    out, _ = kernel_with_results(inputs, trace=False)
    return out
